# revision 67
# baseline (speedup 1.0000x reference)
"""Causal self-attention (B=4, T=2048, C=1024, H=16) on 8 Trainium2 cores.

Sharding: core i handles batch b = i//2 and head-group g = i%2 (8 heads,
512 channels). Each core computes q/k in transposed layout (qkT), v in
natural layout (with a fused ones-column so the attention-times-values
matmul also produces the softmax denominators), causal softmax attention,
and a partial c_proj. The host sums the two head-group partials per batch
and adds the bias row.

Bias folding (exact):
  - k-bias adds a per-query constant to every score -> cancels in softmax.
  - v-bias adds a constant row to y -> contributes (b_v @ W_proj), folded
    into the output bias on the host.
  - q-bias is added on-device when copying qT out of PSUM.

Matmuls run in float32r (full PE rate; ~1e-3 relative precision).
"""

import sys
from contextlib import ExitStack

import numpy as np

sys.path.insert(0, "/opt/trn_rl_repo")

import concourse.bass as bass  # noqa: E402
import concourse.mybir as mybir  # noqa: E402
from concourse.masks import make_identity  # noqa: E402
from concourse.tile import TileContext  # noqa: E402
from concourse.vector_clock import ScopedClock  # noqa: E402

F32 = mybir.dt.float32
F32R = mybir.dt.float32r
BF16 = mybir.dt.bfloat16
EXP = mybir.ActivationFunctionType.Exp
IDENT = mybir.ActivationFunctionType.Identity
IS_GE = mybir.AluOpType.is_ge

B, T, C, H, D = 4, 2048, 1024, 16, 64
NCORES = 8
HL = 8          # heads per core
CL = HL * D     # 512 local channels
KC = C // 128   # 8 contraction chunks
TCH = T // 128  # 16 T chunks of 128
NT = T // 512   # 4 T chunks of 512
SCALE = 1.0 / 8.0  # 1/sqrt(64)


# --------------------------------------------------------------------------
# Workaround: this walrus build accepts only ONE sync-wait per instruction.
# Tile emits several (operand deps, tail drain). Split extras onto fresh
# single-wait EventSemaphore instructions just before each offender on the
# same engine — semantics unchanged, the sequencer blocks on each in turn.
# --------------------------------------------------------------------------
def _split_multiwait_insts(nc):
    ctr = 0
    for f in nc.m.functions:
        for blk in f.blocks:
            insts = list(blk.instructions)
            new_list = []
            changed = False
            for inst in insts:
                si = inst.sync_info
                if si is not None and len(si.on_wait) > 1:
                    waits = list(si.on_wait)
                    keep_idx = len(waits) - 1
                    for i, w in enumerate(waits):
                        if w.wait_reg is not None:
                            keep_idx = i
                            break
                    for i, w in enumerate(waits):
                        if i == keep_idx:
                            continue
                        ev = mybir.InstEventSemaphore(
                            name=f"evsplit_{ctr}", ins=[], outs=[]
                        )
                        ctr += 1
                        ev.engine = inst.engine
                        ev.sync_info = mybir.SyncInfo(on_wait=[w], on_update=[])
                        new_list.append(ev)
                    inst.sync_info.on_wait = [waits[keep_idx]]
                    changed = True
                new_list.append(inst)
            if changed:
                blk.instructions = new_list


def build_bass(repeat=1):
    nc = bass.Bass("TRN2", target_bir_lowering=False, debug=False)

    x_d = nc.dram_tensor("x", [T, C], F32, kind="ExternalInput")
    wqk_d = nc.dram_tensor("wqk", [C, 2 * CL], F32, kind="ExternalInput")
    wv_d = nc.dram_tensor("wv", [C, CL], F32, kind="ExternalInput")
    bq_d = nc.dram_tensor("bq", [4, 128, 1], F32, kind="ExternalInput")
    wp_d = nc.dram_tensor("wp", [CL, C], F32, kind="ExternalInput")
    out_d = nc.dram_tensor("out", [T, C], F32, kind="ExternalOutput")
    scr_d = nc.dram_tensor("scr", [NT * HL, 512], F32)  # recip bounce buffer

    with TileContext(nc) as tc:
        for _rep in range(repeat):
            _emit_body(nc, tc, x_d, wqk_d, wv_d, bq_d, wp_d, out_d, scr_d)

    _split_multiwait_insts(nc)
    return nc


def _emit_body(nc, tc, x_d, wqk_d, wv_d, bq_d, wp_d, out_d, scr_d):
    with ExitStack() as ctx:
        const = ctx.enter_context(tc.tile_pool(name="const", bufs=1))
        big = ctx.enter_context(tc.tile_pool(name="big", bufs=1))

        # persistent tensors
        qkT = big.tile([128, KC, T], F32R)        # rows: q cols (k-tiles 0-3), k cols (4-7)
        vaug = big.tile([128, TCH, HL * 65], F32R)  # per T-chunk: 8x(64 v cols + ones)

        idn = const.tile([128, 128], F32)
        make_identity(nc, idn)
        ones_f = const.tile([1, 64], F32)
        nc.vector.memset(ones_f, 1.0)
        ones_r = const.tile([1, 64], F32R)
        nc.vector.tensor_copy(out=ones_r, in_=ones_f)
        bq_sb = const.tile([128, 4, 1], F32)
        nc.sync.dma_start(out=bq_sb, in_=bq_d.ap().rearrange("a p o -> p a o"))

        # vaug ones-columns are established chunk-by-chunk in the v loop
        # (memset 1.0 then overwrite the v parts), keeping startup unblocked.

        # ------------------------------------------------------------------
        # Phase A: load+round weights, transpose x, QKV matmuls
        # ------------------------------------------------------------------
        with tc.tile_pool(name="wqkr", bufs=1) as wqkr_pool, \
             tc.tile_pool(name="wvr", bufs=1) as wvr_pool, \
             tc.tile_pool(name="stage", bufs=7) as stage, \
             tc.tile_pool(name="xtp", bufs=1) as xtp, \
             tc.tile_pool(name="transps", bufs=4, space="PSUM") as trans_ps, \
             tc.tile_pool(name="mmps", bufs=4, space="PSUM") as mm_ps:

            # quarter-0 x loads first so the transposes (and PE) start
            # immediately; weights go through the Activation HWDGE queue
            xss_pre = []
            for tt in range(4):
                xs = stage.tile([128, 1024], F32, tag="stage")
                nc.sync.dma_start(
                    out=xs, in_=x_d.ap()[tt * 128:(tt + 1) * 128, :]
                )
                xss_pre.append(xs)
            wqk_r = wqkr_pool.tile([128, KC, 2 * CL], F32R)
            wv_r = wvr_pool.tile([128, KC, CL], F32R)
            for k in range(KC):
                ws = stage.tile([128, 1024], F32, tag="stage")
                nc.scalar.dma_start(out=ws, in_=wqk_d.ap()[k * 128:(k + 1) * 128, :])
                nc.vector.tensor_copy(out=wqk_r[:, k, :], in_=ws)
                ws2 = stage.tile([128, CL], F32, tag="stage")
                nc.scalar.dma_start(out=ws2, in_=wv_d.ap()[k * 128:(k + 1) * 128, :])
                nc.vector.tensor_copy(out=wv_r[:, k, :], in_=ws2)

            for quarter in range(4):
                # transpose this quarter of x: xT [128, KC, 512]
                xT = xtp.tile([128, KC, 512], F32R, tag="xT")
                if quarter == 0:
                    xss = xss_pre
                else:
                    xss = []
                    for tt in range(4):  # 128-chunks within quarter
                        tglob = quarter * 4 + tt
                        xs = stage.tile([128, 1024], F32, tag="stage")
                        nc.sync.dma_start(
                            out=xs,
                            in_=x_d.ap()[tglob * 128:(tglob + 1) * 128, :],
                        )
                        xss.append(xs)
                for k in range(KC):
                    tp = trans_ps.tile([128, 512], F32)
                    for tt in range(4):
                        nc.tensor.transpose(
                            out=tp[:, tt * 128:(tt + 1) * 128],
                            in_=xss[tt][:, k * 128:(k + 1) * 128],
                            identity=idn,
                        )
                    nc.vector.tensor_copy(out=xT[:, k, :], in_=tp)
                # qkT columns for this quarter (512 T positions)
                for m in range(8):
                    pq = mm_ps.tile([128, 512], F32, tag="mm")
                    for k in range(KC):
                        nc.tensor.matmul(
                            out=pq,
                            lhsT=wqk_r[:, k, m * 128:(m + 1) * 128],
                            rhs=xT[:, k, :],
                            start=(k == 0),
                            stop=(k == KC - 1),
                        )
                    dst = qkT[:, m, quarter * 512:(quarter + 1) * 512]
                    if m < 4:  # q columns: add q-bias (per-partition)
                        nc.scalar.activation(
                            out=dst, in_=pq, func=IDENT,
                            bias=bq_sb[:, m, :], scale=1.0,
                        )
                    else:
                        nc.vector.tensor_copy(out=dst, in_=pq)
                # v rows for this quarter
                for tt in range(4):
                    tglob = quarter * 4 + tt
                    pv = mm_ps.tile([128, 512], F32, tag="mm")
                    for k in range(KC):
                        nc.tensor.matmul(
                            out=pv,
                            lhsT=xT[:, k, tt * 128:(tt + 1) * 128],
                            rhs=wv_r[:, k, :],
                            start=(k == 0),
                            stop=(k == KC - 1),
                        )
                    # ones for this chunk's per-head denominator columns;
                    # the copy below overwrites the v parts
                    nc.vector.memset(vaug[:, tglob, :].bitcast(F32), 1.0)
                    nc.vector.tensor_copy(
                        out=vaug[:, tglob, :].rearrange(
                            "p (h c) -> p h c", c=65
                        )[:, :, 0:64],
                        in_=pv.rearrange("p (h c) -> p h c", c=64),
                    )

        # ------------------------------------------------------------------
        # Phase B: attention per head, Phase C: partial c_proj
        # ------------------------------------------------------------------
        with tc.tile_pool(name="wpr", bufs=1) as wpr_pool, \
             tc.tile_pool(name="ytp", bufs=1) as ytp, \
             tc.tile_pool(name="stage2", bufs=2) as stage2:

            wp_r = wpr_pool.tile([128, 4, C], F32R)
            for k in range(4):
                ws = stage2.tile([128, 1024], F32, tag="stage2")
                nc.scalar.dma_start(out=ws, in_=wp_d.ap()[k * 128:(k + 1) * 128, :])
                nc.vector.tensor_copy(out=wp_r[:, k, :], in_=ws)

            yT = ytp.tile([128, 4, T], F32R)  # local channels x T

            with tc.tile_pool(name="qkps", bufs=2, space="PSUM") as qk_ps, \
                 tc.tile_pool(name="yps", bufs=3, space="PSUM") as y_ps, \
                 tc.tile_pool(name="projps", bufs=1, space="PSUM") as proj_ps, \
                 tc.tile_pool(name="epool", bufs=3) as e_pool, \
                 tc.tile_pool(name="rcpool", bufs=3) as rc_pool, \
                 tc.tile_pool(name="denpool", bufs=3) as den_pool, \
                 tc.tile_pool(name="mpool", bufs=1) as m_pool, \
                 tc.tile_pool(name="opool", bufs=2) as out_pool:

                # static causal masks, 0/1 bf16: tri = triangular (p <= f)
                # for the restricted j0=2 windows; m0 = both-half shifted
                # mask for the full-width j0=0 pair
                tri = m_pool.tile([128, 512], BF16, tag="tri")
                nc.vector.memset(tri, 1.0)
                nc.gpsimd.affine_select(
                    out=tri, in_=tri, pattern=[[1, 512]],
                    compare_op=IS_GE, fill=0.0,
                    base=0, channel_multiplier=-1,
                )
                m0 = m_pool.tile([128, 2, 512], BF16, tag="m0")
                nc.vector.memset(m0, 1.0)
                nc.gpsimd.affine_select(
                    out=m0, in_=m0, pattern=[[-128, 2], [1, 512]],
                    compare_op=IS_GE, fill=0.0,
                    base=0, channel_multiplier=-1,
                )
                m0f = m0.rearrange("p a f -> p (a f)")

                def emit_proj_tq(tq, act=False):
                    # projection for one 128-row T chunk. act=True drains
                    # PSUM via the Activation engine — used for the tail
                    # projections, where DVE is busy with the short attention
                    # chunks' masks/normalize but ACT has run out of exps.
                    os_ = out_pool.tile([128, 1024], F32, tag="os")
                    for oc in range(2):
                        pp = proj_ps.tile([128, 512], F32, tag="proj")
                        for k in range(4):
                            nc.tensor.matmul(
                                out=pp,
                                lhsT=yT[:, k, tq * 128:(tq + 1) * 128],
                                rhs=wp_r[:, k, oc * 512:(oc + 1) * 512],
                                start=(k == 0),
                                stop=(k == 3),
                            )
                        if act:
                            nc.scalar.copy(
                                out=os_[:, oc * 512:(oc + 1) * 512], in_=pp
                            )
                        else:
                            nc.vector.tensor_copy(
                                out=os_[:, oc * 512:(oc + 1) * 512], in_=pp
                            )
                    nc.sync.dma_start(
                        out=out_d.ap()[tq * 128:(tq + 1) * 128, :], in_=os_
                    )

                def att_group(c, l):
                    row = (l % 2) * 64
                    qtile = l // 2
                    ktile = 4 + l // 2
                    if True:
                        yps = y_ps.tile([128, 512], F32, tag="yps")
                        npairs = 2 * c + 2  # tk pairs (2 blocks each)
                        for pj in range(npairs):
                            j0 = 2 * pj - 4 * c
                            pqk = qk_ps.tile([128, 1024], F32, tag="qk")
                            e = e_pool.tile([128, 1024], F32R, tag="e")
                            # valid column window per half for the second
                            # diagonal pair: block j keeps cols >= 128*j.
                            los = {2: (256, 384)}.get(j0, None)
                            for h in range(2):
                                tk = 2 * pj + h
                                lo = los[h] if los else 0
                                nc.tensor.matmul(
                                    out=pqk[:, h * 512 + lo:(h + 1) * 512],
                                    lhsT=qkT[row:row + 64, ktile,
                                             tk * 128:(tk + 1) * 128],
                                    rhs=qkT[row:row + 64, qtile,
                                            c * 512 + lo:(c + 1) * 512],
                                    start=True,
                                    stop=True,
                                )
                            if los:
                                for h in range(2):
                                    lo = los[h]
                                    sl = slice(h * 512 + lo, (h + 1) * 512)
                                    nc.scalar.activation(
                                        out=e[:, sl], in_=pqk[:, sl],
                                        func=EXP, scale=SCALE,
                                    )
                                    # causal: p <= f_local within the window
                                    nc.vector.tensor_mul(
                                        e[:, sl], e[:, sl], tri[:, 0:512 - lo]
                                    )
                            else:
                                nc.scalar.activation(
                                    out=e, in_=pqk, func=EXP, scale=SCALE
                                )
                                if j0 == 0:  # diagonal: full-width mask
                                    nc.vector.tensor_mul(e, e, m0f)
                            for h in range(2):
                                tk = 2 * pj + h
                                lo = los[h] if los else 0
                                nc.tensor.matmul(
                                    out=yps[0:65, lo:512],
                                    lhsT=vaug[:, tk, l * 65:(l + 1) * 65],
                                    rhs=e[:, h * 512 + lo:(h + 1) * 512],
                                    start=(tk == 0),
                                    stop=(tk == 2 * npairs - 1),
                                )
                        # normalize: row 64 of yps holds the softmax
                        # denominators. Reciprocal -> DRAM bounce -> stride-0
                        # broadcast load -> multiply (no PE involvement).
                        idx = c * HL + l
                        r = rc_pool.tile([1, 512], F32R, tag="rc")
                        with nc.allow_low_precision(reason="softmax reciprocal f32r"):
                            nc.vector.reciprocal(out=r, in_=yps[64:65, :])
                        nc.sync.dma_start(
                            out=scr_d.ap()[idx:idx + 1, :],
                            in_=r.bitcast(F32),
                        )
                        den = den_pool.tile([64, 512], F32R, tag="den")
                        nc.sync.dma_start(
                            out=den.bitcast(F32),
                            in_=scr_d.ap()[idx:idx + 1, :].to_broadcast((64, 512)),
                        )
                        nc.vector.tensor_mul(
                            yT[row:row + 64, qtile, c * 512:(c + 1) * 512],
                            yps[0:64, :],
                            den,
                        )

                # Big chunks first; each chunk's projection is emitted one
                # chunk later so its PE-dense work fills the next (shorter)
                # chunk's latency stalls.
                pending = None
                order = (3, 1, 2, 0)
                for ci, c in enumerate(order):
                    for l in range(HL):
                        att_group(c, l)
                    if pending is not None:
                        # the last filler runs beside att(0): ACT drains
                        for tq in range(4 * pending, 4 * pending + 4):
                            emit_proj_tq(tq, act=(ci == len(order) - 1))
                    pending = c
                for tq in range(4 * pending, 4 * pending + 4):
                    emit_proj_tq(tq, act=True)


# --------------------------------------------------------------------------
# Cached PJRT execution (mirrors bass2jax.run_bass_via_pjrt but reuses the
# compiled executable across kernel() calls).
# --------------------------------------------------------------------------
_CACHE = {}


def _get_runner(repeat=1):
    key = ("runner", repeat)
    if key in _CACHE:
        return _CACHE[key]

    import jax
    from jax.sharding import Mesh, PartitionSpec
    from jax.experimental.shard_map import shard_map
    from concourse import bass2jax

    nc = build_bass(repeat=repeat)
    bass2jax.install_neuronx_cc_hook()

    partition_name = (
        nc.partition_id_tensor.name if nc.partition_id_tensor else None
    )
    in_names, out_names, out_avals, zero_shapes = [], [], [], []
    for alloc in nc.m.functions[0].allocations:
        if not isinstance(alloc, mybir.MemoryLocationSet):
            continue
        name = alloc.memorylocations[0].name
        if alloc.kind == "ExternalInput":
            if name != partition_name:
                in_names.append(name)
        elif alloc.kind == "ExternalOutput":
            shape = tuple(alloc.tensor_shape)
            dtype = mybir.dt.np(alloc.dtype)
            out_names.append(name)
            out_avals.append(jax.core.ShapedArray(shape, dtype))
            zero_shapes.append((shape, dtype))
    n_params = len(in_names)
    n_outs = len(out_avals)
    all_in_names = list(in_names) + list(out_names)
    if partition_name is not None:
        all_in_names.append(partition_name)

    def _body(*args):
        operands = list(args)
        if partition_name is not None:
            operands.append(bass2jax.partition_id_tensor())
        outs = bass2jax._bass_exec_p.bind(
            *operands,
            out_avals=tuple(out_avals),
            in_names=tuple(all_in_names),
            out_names=tuple(out_names),
            lowering_input_output_aliases=(),
            sim_require_finite=True,
            sim_require_nnan=True,
            nc=nc,
        )
        return tuple(outs)

    devices = jax.devices()[:NCORES]
    mesh = Mesh(np.asarray(devices), ("core",))
    in_specs = (PartitionSpec("core"),) * (n_params + n_outs)
    out_specs = (PartitionSpec("core"),) * n_outs
    donate = tuple(range(n_params, n_params + n_outs))
    sharded = jax.jit(
        shard_map(
            _body, mesh=mesh, in_specs=in_specs, out_specs=out_specs,
            check_rep=False,
        ),
        donate_argnums=donate,
        keep_unused=True,
    )

    runner = {
        "sharded": sharded,
        "in_names": in_names,
        "out_names": out_names,
        "zero_shapes": zero_shapes,
        "n_params": n_params,
        "mesh": mesh,
    }
    _CACHE[key] = runner
    return runner


def _make_core_inputs(x, W_attn, b_attn, W_proj):
    """Per-core input dicts (core i: batch i//2, head-group i%2)."""
    x = np.ascontiguousarray(x, dtype=np.float32)
    W_attn = np.ascontiguousarray(W_attn, dtype=np.float32)
    b_attn = np.ascontiguousarray(b_attn, dtype=np.float32)
    W_proj = np.ascontiguousarray(W_proj, dtype=np.float32)

    per_group = []
    for g in range(2):
        s = g * CL
        wqk = np.ascontiguousarray(
            np.concatenate(
                [W_attn[:, s:s + CL], W_attn[:, C + s:C + s + CL]], axis=1
            )
        )
        wv = np.ascontiguousarray(W_attn[:, 2 * C + s:2 * C + s + CL])
        bq = np.ascontiguousarray(b_attn[s:s + CL].reshape(4, 128, 1))
        wp = np.ascontiguousarray(W_proj[s:s + CL, :])
        per_group.append((wqk, wv, bq, wp))

    in_maps = []
    for core in range(NCORES):
        b_i, g = core // 2, core % 2
        wqk, wv, bq, wp = per_group[g]
        in_maps.append(
            {"x": np.ascontiguousarray(x[b_i]), "wqk": wqk, "wv": wv,
             "bq": bq, "wp": wp}
        )
    return in_maps


def run_cores(in_maps, timing_reps=0, repeat=1):
    """Run the SPMD kernel. Returns (list of per-core output dicts, best_ns).

    timing_reps > 0 additionally re-executes the cached executable on
    device-resident inputs and reports the best wall-clock per call in ns.
    """
    import jax, time

    r = _get_runner(repeat=repeat)
    per_core = [
        [np.asarray(m[name]) for name in r["in_names"]] for m in in_maps
    ]
    concat_in = [
        np.concatenate([per_core[c][i] for c in range(NCORES)], axis=0)
        for i in range(len(r["in_names"]))
    ]
    def zeros():
        return [
            np.zeros((NCORES * s[0], *s[1:]), dt) for (s, dt) in r["zero_shapes"]
        ]

    out_arrs = r["sharded"](*concat_in, *zeros())
    outs_np = [np.asarray(a) for a in out_arrs]

    best_ns = None
    if timing_reps > 0:
        from jax.sharding import NamedSharding, PartitionSpec

        shard = NamedSharding(r["mesh"], PartitionSpec("core"))
        dev_in = [jax.device_put(a, shard) for a in concat_in]
        for a in dev_in:
            a.block_until_ready()
        # pre-stage one donated zero-set per timed call (donation consumes them)
        zsets = []
        for _ in range(timing_reps + 1):
            zs = [jax.device_put(z, shard) for z in zeros()]
            for a in zs:
                a.block_until_ready()
            zsets.append(zs)
        res = r["sharded"](*dev_in, *zsets[0])  # warm
        for a in res:
            a.block_until_ready()
        times = []
        for i in range(timing_reps):
            t0 = time.perf_counter()
            res = r["sharded"](*dev_in, *zsets[i + 1])
            for a in res:
                a.block_until_ready()
            t1 = time.perf_counter()
            times.append(t1 - t0)
        best_ns = int(min(times) * 1e9)

    results = []
    for c in range(NCORES):
        m = {}
        for i, name in enumerate(r["out_names"]):
            full = outs_np[i]
            shape = r["zero_shapes"][i][0]
            m[name] = full.reshape(NCORES, *shape)[c]
        results.append(m)
    return results, best_ns


def kernel(x, W_attn, b_attn, W_proj, b_proj, _timing_reps=0, _return_ns=False):
    x = np.asarray(x, dtype=np.float32)
    W_attn = np.asarray(W_attn, dtype=np.float32)
    b_attn = np.asarray(b_attn, dtype=np.float32)
    W_proj = np.asarray(W_proj, dtype=np.float32)
    b_proj = np.asarray(b_proj, dtype=np.float32)

    in_maps = _make_core_inputs(x, W_attn, b_attn, W_proj)
    results, best_ns = run_cores(in_maps, timing_reps=_timing_reps)

    # v-bias contributes a constant row through the projection
    bias_row = (b_proj + b_attn[2 * C:3 * C] @ W_proj).astype(np.float32)

    out = np.empty((B, T, C), dtype=np.float32)
    for b_i in range(B):
        out[b_i] = results[2 * b_i]["out"] + results[2 * b_i + 1]["out"]
        out[b_i] += bias_row[None, :]
    if _return_ns:
        return out, best_ns
    return out
